# revision 1
# baseline (speedup 1.0000x reference)
"""Adaptive polyphase sampling (stride 2, p=2) on 8 TRN2 NeuronCores.

For x [32, 256, 64, 64] f32: compute the 4 polyphase components
x[:, :, i::2, j::2], pick per-sample the component with the largest L2
norm (over channels+space), return it [32, 256, 32, 32].

Sharding: pure data parallel over batch — 4 samples per core, no
cross-core communication.

Layout: partition p holds the channel pair {2p, 2p+1}; each sample is
one contiguous 32 KiB run per partition.

v2 design — select by engine branch + single copy:
  v1 materialized the selection as a 4-step mask-multiply chain on the
  vector engine (~34us of DVE work and a 26us serial tail). Here the
  argmax is resolved into *engine registers* and the selection is a
  CONTROL-FLOW decision: the vector engine TENSOR_LOADs the 4 norm
  totals (non-negative f32 bit patterns compare correctly as int32),
  computes their max with 3 reg ALU ops, and walks an If/Else
  compare-branch chain; the taken arm runs ONE copy (fp32 COPY runs at
  2x: ~1.2us/sample) of the chosen component into a contiguous obuf,
  which a contiguous DMA then writes out. Ties resolve to the lowest
  k, matching jnp.argmax.

  norms: scalar does k=0,1 (ACT Square + accum_out), vector does k=2,3
  (scalar_tensor_tensor in*in with accum_out). Per-partition partials
  land in npart; a ones[128,128] matmul sums them across partitions
  into PSUM (sample 3's four chunk-partial sets accumulate in one PSUM
  bank); a tiny vector copy drops psum row 0 into nsum for the
  TENSOR_LOADs.

  DMA plan: reads and writes share the per-NC HBM pipe (~358 GB/s), so
  total wire time is fixed at ~21MB/rate and the optimal schedule is
  ALL reads first, then all writes. Everything rides the sync HWDGE
  ring whose FIFO guarantees exactly that order: 7 input DMAs (samples
  0-2 whole, sample 3 in 4 chunks so its norms pipeline behind the
  stream), then out0/out1/out2/out3-plane0 as each selection copy
  lands. Sample 3's decision + copy (~2.5us) hides entirely under the
  s0-s2 write drain; only its plane-1 output uses the scalar HWDGE
  ring so the two final 0.5MB writes drain concurrently.

"""

from contextlib import ExitStack

import numpy as np

import concourse.bass as bass
from concourse import mybir
from concourse.bass_utils import run_bass_kernel_spmd

F32 = mybir.dt.float32
I32 = mybir.dt.int32
OP = mybir.AluOpType
ACT = mybir.ActivationFunctionType

B, C, H, W = 32, 256, 64, 64
NCORES = 8
SPC = B // NCORES          # samples per core
H2, W2 = H // 2, W // 2    # 32, 32
SP = H * W                 # 4096 spatial elems per channel
OSP = H2 * W2              # 1024
LAST = SPC - 1

NCHUNK = 4                 # input chunks for the last sample
CSP = SP // NCHUNK         # 1024 spatial elems per chunk
CR = H2 // NCHUNK          # 8 r-rows per chunk

# ---- static semaphore plan ----------------------------------------------
# vector chain (vch), in vector program order
VCH = {}
_c = 0
for _ev, _n in [("n0", 2), ("cp0", 1), ("sel0", 1), ("n1", 2), ("cp1", 1),
                ("sel1", 1), ("n2", 2), ("nc0", 2), ("cp2", 1), ("sel2", 1),
                ("nc1", 2), ("nc2", 2), ("nc3", 2), ("cp3", 1), ("sel3", 1)]:
    _c += _n
    VCH[_ev] = _c
VCH_TOTAL = _c

# scalar chain (sch)
SCH = {}
_c = 0
for _ev, _n in [("pre", 1), ("n0", 2), ("n1", 2), ("n2", 2), ("nc0", 2),
                ("nc1", 2), ("nc2", 2), ("nc3", 2)]:
    _c += _n
    SCH[_ev] = _c
SCH_TOTAL = _c

# matmul sem: s0, s1, s2 then the 4 chunk accumulations for s3
MM = {"s0": 1, "s1": 2, "s2": 3, "s3": 3 + NCHUNK}


def build_nc():
    nc = bass.Bass("TRN2", target_bir_lowering=False, debug=False)
    x = nc.dram_tensor("x", [SPC, C, H, W], F32, kind="ExternalInput")
    out = nc.dram_tensor("out", [SPC, C, H2, W2], F32, kind="ExternalOutput")

    x_aps = [
        x.ap()[s].rearrange("(p c) h w -> p c (h w)", c=2) for s in range(SPC)
    ]
    out_aps = [
        out.ap()[s].rearrange("(p c) a b -> p c (a b)", c=2) for s in range(SPC)
    ]

    with ExitStack() as ctx:
        block = ctx.enter_context(nc.Block(no_gpsimd_drain=True))
        sem = lambda name: ctx.enter_context(nc.semaphore(name))
        sb = lambda name, shape: ctx.enter_context(nc.sbuf_tensor(name, shape, F32))
        dmains = [sem(f"dmain{i}") for i in range(SPC - 1)]
        dmain3s = [sem(f"dmain3_{c}") for c in range(NCHUNK)]
        # output-completion sems: s0, s1, s2, s3-plane0, s3-plane1
        douts = [sem(f"dout{i}") for i in range(SPC + 1)]
        sch, vch, gch, mm = sem("sch"), sem("vch"), sem("gch"), sem("mm")
        samps = [sb(f"samp{i}", [128, 2, SP]) for i in range(SPC)]
        obufs = [sb(f"obuf{i}", [128, 2, OSP]) for i in range(SPC)]
        # DVE main-out sinks, one per k so the two TTRs of a norm group
        # don't collide; groups are separated by a vch self-barrier
        dumps = [sb(f"dump{i}", [128, 2 * OSP]) for i in range(2)]
        sdump = sb("sdump", [128, 8])
        # per-partition norm partials: s0-2 at cols 4s+k; s3 at 16+k*4+c
        npart = sb("npart", [128, 32])
        nsum = sb("nsum", [128, 4 * SPC])        # broadcast totals (row 0)
        ones = sb("ones", [128, 128])
        psums = [
            ctx.enter_context(nc.psum_tensor(f"ps{i}", [128, 4], F32))
            for i in range(SPC)
        ]

        # 3D component views (walrus wants <=3D DVE APs): per partition the
        # flat 8192-elem sample layout is a*128 + i*64 + q*2 + j with
        # a = c*32 + r in [0, 64). Chunks cover flat quarters a in [16c,
        # 16c+16) — contiguous in both DRAM and SBUF.
        def V(s, k, a0=0, a1=2 * H2, plane=None):
            i, j = divmod(k, 2)
            if plane is not None:
                a0, a1 = plane * H2, (plane + 1) * H2
            return bass.AP(
                samps[s], i * W + j + a0 * 2 * W,
                [[2 * SP, 128], [2 * W, a1 - a0], [2, W2]],
            )

        def OB(s, a0=0, a1=2 * H2, plane=None):
            if plane is not None:
                a0, a1 = plane * H2, (plane + 1) * H2
            return bass.AP(
                obufs[s], a0 * W2, [[2 * OSP, 128], [W2, a1 - a0], [1, W2]]
            )

        def Vc(s, k, c):
            return V(s, k, 16 * c, 16 * (c + 1))

        zsink = lambda n: bass.AP(sdump, 0, [[8, 128], [0, n], [0, W2]])
        np_col = lambda c: npart.ap()[:, c : c + 1]
        ncol_s = lambda s, k: np_col(4 * s + k)
        ncol_c = lambda k, c: np_col(16 + 4 * k + c)
        nsum_i32 = lambda s: nsum.ap()[0:1, 4 * s : 4 * s + 4].bitcast(I32)

        def load_max(eng, s, rr):
            """Load the 4 norm totals of sample s into rr[0..3]; rr[4]=max."""
            eng.reg_load(rr[0:4], nsum_i32(s))
            eng.reg_alu(rr[4], rr[0], rr[1], OP.max)
            eng.reg_alu(rr[4], rr[4], rr[2], OP.max)
            eng.reg_alu(rr[4], rr[4], rr[3], OP.max)

        def branch_select(eng, rr, arm):
            """arm(k) emits the taken component's op; exactly one arm runs;
            ties take the lowest k (matches jnp.argmax)."""
            m = eng.snap(rr[4], min_val=-(2**31), max_val=2**31 - 1)
            with eng.If_cmp(rr[0], m, "IS_EQ"):
                arm(0)
            with eng.Else():
                with eng.If_cmp(rr[1], m, "IS_EQ"):
                    arm(1)
                with eng.Else():
                    with eng.If_cmp(rr[2], m, "IS_EQ"):
                        arm(2)
                    with eng.Else():
                        arm(3)

        @block.gpsimd
        def _(gpsimd):
            gpsimd.memset(ones.ap(), 1.0).then_inc(gch, 1)

        @block.sync
        def _(sync):
            # inputs: samples 0-2 whole, sample 3 in NCHUNK chunks
            for s in range(SPC - 1):
                sync.dma_start(out=samps[s].ap(), in_=x_aps[s]).then_inc(
                    dmains[s], 16
                )
            for c in range(NCHUNK):
                ch, off = c // 2, (c % 2) * (SP // 2)
                sl = slice(off, off + SP // 2)
                sync.dma_start(
                    out=samps[LAST].ap()[:, ch : ch + 1, sl],
                    in_=x_aps[LAST][:, ch : ch + 1, sl],
                ).then_inc(dmain3s[c], 16)

            # outputs ride the sync ring's FIFO: their descriptors queue
            # behind the input stream, so reads are never preempted by
            # writes (R+W share the per-NC HBM pipe; interleaving writes
            # only delays the last read and with it sample 3's decision)
            for s in range(SPC - 1):
                sync.wait_ge(vch, VCH[f"sel{s}"])
                sync.dma_start(out=out_aps[s], in_=obufs[s].ap()).then_inc(
                    douts[s], 16
                )
            sync.wait_ge(vch, VCH["sel3"])
            sync.dma_start(
                out=out_aps[LAST][:, 0], in_=obufs[LAST].ap()[:, 0]
            ).then_inc(douts[3], 16)

            for i in range(SPC + 1):
                sync.wait_ge(douts[i], 16)

        @block.tensor
        def _(tensor):
            tensor.wait_ge(gch, 1)
            for s in range(SPC - 1):
                tensor.wait_ge(sch, SCH[f"n{s}"])
                tensor.wait_ge(vch, VCH[f"n{s}"])
                tensor.matmul(
                    psums[s].ap(),
                    ones.ap(),
                    npart.ap()[:, 4 * s : 4 * s + 4],
                    start=True,
                    stop=True,
                ).then_inc(mm, 1)
            mv = npart.ap()[:, 16:32].rearrange("p (k c) -> p c k", c=NCHUNK)
            for c in range(NCHUNK):
                tensor.wait_ge(sch, SCH[f"nc{c}"])
                tensor.wait_ge(vch, VCH[f"nc{c}"])
                tensor.matmul(
                    psums[LAST].ap(),
                    ones.ap(),
                    mv[:, c],
                    start=(c == 0),
                    stop=(c == NCHUNK - 1),
                ).then_inc(mm, 1)

        @block.scalar
        def _(scalar):
            cnt = [0]

            def emit(inst):
                inst.then_inc(sch, 1)
                cnt[0] += 1

            def barrier():
                if cnt[0]:
                    scalar.wait_ge(sch, cnt[0])

            # preload the Square activation table before any data arrives
            emit(
                scalar.activation(
                    sdump.ap()[:, 0:1], sdump.ap()[:, 0:1], ACT.Square, scale=0.0
                )
            )

            # norms k=0,1; each activation lowers to ACT + READ_ACCUM
            for s in range(SPC - 1):
                scalar.wait_ge(dmains[s], 16)
                for k in (0, 1):
                    barrier()
                    emit(
                        scalar.activation(
                            zsink(2 * H2), V(s, k), ACT.Square,
                            accum_out=ncol_s(s, k),
                        )
                    )
            for c in range(NCHUNK):
                scalar.wait_ge(dmain3s[c], 16)
                for k in (0, 1):
                    barrier()
                    emit(
                        scalar.activation(
                            zsink(16), Vc(LAST, k, c), ACT.Square,
                            accum_out=ncol_c(k, c),
                        )
                    )

            # s3 plane 1 output rides the scalar ring (vector copies the
            # whole sample; an ACT-copy here would trigger a conservative
            # ACT_TABLE_LOAD inside the branch arms, right in the tail)
            scalar.wait_ge(vch, VCH["sel3"])
            scalar.dma_start(
                out=out_aps[LAST][:, 1], in_=obufs[LAST].ap()[:, 1]
            ).then_inc(douts[4], 16)

        @block.vector
        def _(vector):
            cnt = [0]

            def emit(inst):
                inst.then_inc(vch, 1)
                cnt[0] += 1

            def barrier():
                if cnt[0]:
                    vector.wait_ge(vch, cnt[0])

            regs = [ctx.enter_context(vector.register(f"ve_r{i}"))
                    for i in range(5)]

            def ttr(out_sink, in_, acc):
                # out = (in*1.0)*in = in^2, accum_out = per-partition sum
                emit(
                    vector.scalar_tensor_tensor(
                        out=out_sink, in0=in_, scalar=1.0, in1=in_,
                        op0=OP.mult, op1=OP.mult, accum_out=acc,
                    )
                )

            def dsink(k, n):
                return bass.AP(dumps[k - 2], 0, [[2 * OSP, 128], [W2, n], [1, W2]])

            def nrm(s):
                vector.wait_ge(dmains[s], 16)
                barrier()  # prior group's writes to the dump sinks
                for k in (2, 3):
                    ttr(dsink(k, 2 * H2), V(s, k), ncol_s(s, k))

            def nrm_c(c):
                vector.wait_ge(dmain3s[c], 16)
                barrier()
                for k in (2, 3):
                    ttr(dsink(k, 16), Vc(LAST, k, c), ncol_c(k, c))

            def cp(s):
                vector.wait_ge(mm, MM[f"s{s}"])
                emit(
                    vector.tensor_copy(
                        nsum.ap()[0:1, 4 * s : 4 * s + 4], psums[s].ap()[0:1]
                    )
                )

            def sel(s, plane=None):
                barrier()  # nsum must be drained before the TENSOR_LOAD
                load_max(vector, s, regs)

                def arm(k):
                    # exactly one arm runs at runtime; +1 on vch either way
                    vector.tensor_copy(
                        OB(s, plane=plane), V(s, k, plane=plane)
                    ).then_inc(vch, 1)

                branch_select(vector, regs, arm)
                cnt[0] += 1
                assert cnt[0] == VCH[f"sel{s}"]

            nrm(0)
            cp(0)
            sel(0)
            nrm(1)
            cp(1)
            sel(1)
            nrm(2)
            nrm_c(0)
            cp(2)
            sel(2)
            for c in range(1, NCHUNK):
                nrm_c(c)
            cp(3)
            sel(3)  # plane 1 is scalar's
            assert cnt[0] == VCH_TOTAL

    return nc


_NC_CACHE = None


def _get_nc():
    global _NC_CACHE
    if _NC_CACHE is None:
        _NC_CACHE = build_nc()
    return _NC_CACHE


def _ensure_devices():
    """Best-effort: make sure the axon NeuronCore backend is selected even if
    the caller initialized jax with a CPU-only platform."""
    import jax

    try:
        if len(jax.devices()) >= NCORES:
            return
    except Exception:
        pass
    try:
        jax.config.update("jax_platforms", "axon")
    except Exception:
        pass


def kernel(x) -> np.ndarray:
    _ensure_devices()
    x = np.asarray(x, dtype=np.float32)
    assert x.shape == (B, C, H, W), x.shape
    shards = np.split(x, NCORES, axis=0)
    in_maps = [{"x": s} for s in shards]
    res = run_bass_kernel_spmd(_get_nc(), in_maps, core_ids=list(range(NCORES)))
    return np.concatenate([r["out"] for r in res.results], axis=0)



# revision 10
# speedup vs baseline: 1.0304x; 1.0304x over previous
"""Adaptive polyphase sampling (stride 2, p=2) on 8 TRN2 NeuronCores.

For x [32, 256, 64, 64] f32: compute the 4 polyphase components
x[:, :, i::2, j::2], pick per-sample the component with the largest L2
norm (over channels+space), return it [32, 256, 32, 32].

Sharding: pure data parallel over batch — 4 samples per core, no
cross-core communication.

Layout: partition p holds the channel pair {2p, 2p+1}; each sample is
one contiguous 32 KiB run per partition.

v2 design — select by engine branch + single copy:
  v1 materialized the selection as a 4-step mask-multiply chain on the
  vector engine (~34us of DVE work and a 26us serial tail). Here the
  argmax is resolved into *engine registers* and the selection is a
  CONTROL-FLOW decision: the vector engine TENSOR_LOADs the 4 norm
  totals (non-negative f32 bit patterns compare correctly as int32),
  computes their max with 3 reg ALU ops, and walks an If/Else
  compare-branch chain; the taken arm runs ONE copy (fp32 COPY runs at
  2x: ~1.2us/sample) of the chosen component into a contiguous obuf,
  which a contiguous DMA then writes out. Ties resolve to the lowest
  k, matching jnp.argmax.

  norms: scalar does k=0,1 (ACT Square + accum_out), vector does k=2,3
  (scalar_tensor_tensor in*in with accum_out). Per-partition partials
  land in npart; a ones[128,128] matmul sums them across partitions
  into PSUM (sample 3's four chunk-partial sets accumulate in one PSUM
  bank); a tiny vector copy drops psum row 0 into nsum for the
  TENSOR_LOADs.

  DMA plan (v3): reads stream on the sync HWDGE ring (samples 0-2
  whole, sample 3 in 4 chunks so its norms pipeline behind the
  stream). Writes are split across both HWDGE rings and mostly issued
  EARLY: out0/out1 go out on the scalar ring as soon as their
  selections land (~22us/~34us), overlapping the read stream, because
  all 8 cores otherwise bunch their 4.2MB write phases into the same
  post-read wall-clock window, oversubscribing chip HBM — and core 0
  (first dispatched) loses that arbitration hard (~176 GB/s crawl,
  +10us on the straggler). Only out2 + out3-plane0 stay queued at the
  sync ring's tail — enough write drain to hide sample 3's decision +
  copy (~2.5us), small enough to cap tail-contention exposure; out3's
  plane 1 drains concurrently on the scalar ring. Completion is
  tracked per ring tail only (doutA/doutB): HWDGE rings are per-engine
  FIFO, so the tail DMA's semaphore implies all earlier writes landed.

"""

from contextlib import ExitStack

import numpy as np

import concourse.bass as bass
from concourse import mybir
from concourse.bass_utils import run_bass_kernel_spmd

F32 = mybir.dt.float32
I32 = mybir.dt.int32
OP = mybir.AluOpType
ACT = mybir.ActivationFunctionType

B, C, H, W = 32, 256, 64, 64
NCORES = 8
SPC = B // NCORES          # samples per core
H2, W2 = H // 2, W // 2    # 32, 32
SP = H * W                 # 4096 spatial elems per channel
OSP = H2 * W2              # 1024
LAST = SPC - 1

NCHUNK = 4                 # input chunks for the last sample
CSP = SP // NCHUNK         # 1024 spatial elems per chunk
CR = H2 // NCHUNK          # 8 r-rows per chunk

# ---- static semaphore plan ----------------------------------------------
# vector chain (vch), in vector program order
VCH = {}
_c = 0
for _ev, _n in [("n0", 2), ("cp0", 1), ("sel0", 1), ("n1", 2), ("cp1", 1),
                ("sel1", 1), ("n2", 2), ("nc0", 2), ("cp2", 1), ("sel2", 1),
                ("nc1", 2), ("nc2", 2), ("nc3", 2), ("cp3", 1), ("sel3", 1)]:
    _c += _n
    VCH[_ev] = _c
VCH_TOTAL = _c

# scalar chain (sch)
SCH = {}
_c = 0
for _ev, _n in [("pre", 1), ("n0", 2), ("n1", 2), ("n2", 2), ("nc0", 2),
                ("nc1", 2), ("nc2", 2), ("nc3", 2)]:
    _c += _n
    SCH[_ev] = _c
SCH_TOTAL = _c

# matmul sem: s0, s1, s2 then the 4 chunk accumulations for s3
MM = {"s0": 1, "s1": 2, "s2": 3, "s3": 3 + NCHUNK}


def build_nc():
    nc = bass.Bass("TRN2", target_bir_lowering=False, debug=False)
    x = nc.dram_tensor("x", [SPC, C, H, W], F32, kind="ExternalInput")
    out = nc.dram_tensor("out", [SPC, C, H2, W2], F32, kind="ExternalOutput")

    x_aps = [
        x.ap()[s].rearrange("(p c) h w -> p c (h w)", c=2) for s in range(SPC)
    ]
    out_aps = [
        out.ap()[s].rearrange("(p c) a b -> p c (a b)", c=2) for s in range(SPC)
    ]

    with ExitStack() as ctx:
        block = ctx.enter_context(nc.Block(no_gpsimd_drain=True))
        sem = lambda name: ctx.enter_context(nc.semaphore(name))
        sb = lambda name, shape: ctx.enter_context(nc.sbuf_tensor(name, shape, F32))
        dmains = [sem(f"dmain{i}") for i in range(SPC - 1)]
        dmain3s = [sem(f"dmain3_{c}") for c in range(NCHUNK)]
        # one completion sem per HWDGE ring tail: per-engine FIFO within a
        # ring means the tail DMA's completion implies all earlier ones
        doutA, doutB = sem("doutA"), sem("doutB")
        sch, vch, gch, mm = sem("sch"), sem("vch"), sem("gch"), sem("mm")
        samps = [sb(f"samp{i}", [128, 2, SP]) for i in range(SPC)]
        obufs = [sb(f"obuf{i}", [128, 2, OSP]) for i in range(SPC)]
        # DVE main-out sinks, one per k so the two TTRs of a norm group
        # don't collide; groups are separated by a vch self-barrier
        dumps = [sb(f"dump{i}", [128, 2 * OSP]) for i in range(2)]
        sdump = sb("sdump", [128, 8])
        # per-partition norm partials: s0-2 at cols 4s+k; s3 at 16+k*4+c
        npart = sb("npart", [128, 32])
        nsum = sb("nsum", [128, 4 * SPC])        # broadcast totals (row 0)
        ones = sb("ones", [128, 128])
        psums = [
            ctx.enter_context(nc.psum_tensor(f"ps{i}", [128, 4], F32))
            for i in range(SPC)
        ]

        # 3D component views (walrus wants <=3D DVE APs): per partition the
        # flat 8192-elem sample layout is a*128 + i*64 + q*2 + j with
        # a = c*32 + r in [0, 64). Chunks cover flat quarters a in [16c,
        # 16c+16) — contiguous in both DRAM and SBUF.
        def V(s, k, a0=0, a1=2 * H2, plane=None):
            i, j = divmod(k, 2)
            if plane is not None:
                a0, a1 = plane * H2, (plane + 1) * H2
            return bass.AP(
                samps[s], i * W + j + a0 * 2 * W,
                [[2 * SP, 128], [2 * W, a1 - a0], [2, W2]],
            )

        def OB(s, a0=0, a1=2 * H2, plane=None):
            if plane is not None:
                a0, a1 = plane * H2, (plane + 1) * H2
            return bass.AP(
                obufs[s], a0 * W2, [[2 * OSP, 128], [W2, a1 - a0], [1, W2]]
            )

        def Vc(s, k, c):
            return V(s, k, 16 * c, 16 * (c + 1))

        zsink = lambda n: bass.AP(sdump, 0, [[8, 128], [0, n], [0, W2]])
        np_col = lambda c: npart.ap()[:, c : c + 1]
        ncol_s = lambda s, k: np_col(4 * s + k)
        ncol_c = lambda k, c: np_col(16 + 4 * k + c)
        nsum_i32 = lambda s: nsum.ap()[0:1, 4 * s : 4 * s + 4].bitcast(I32)

        def load_max(eng, s, rr):
            """Load the 4 norm totals of sample s into rr[0..3]; rr[4]=max."""
            eng.reg_load(rr[0:4], nsum_i32(s))
            eng.reg_alu(rr[4], rr[0], rr[1], OP.max)
            eng.reg_alu(rr[4], rr[4], rr[2], OP.max)
            eng.reg_alu(rr[4], rr[4], rr[3], OP.max)

        def branch_select(eng, rr, arm):
            """arm(k) emits the taken component's op; exactly one arm runs;
            ties take the lowest k (matches jnp.argmax)."""
            m = eng.snap(rr[4], min_val=-(2**31), max_val=2**31 - 1)
            with eng.If_cmp(rr[0], m, "IS_EQ"):
                arm(0)
            with eng.Else():
                with eng.If_cmp(rr[1], m, "IS_EQ"):
                    arm(1)
                with eng.Else():
                    with eng.If_cmp(rr[2], m, "IS_EQ"):
                        arm(2)
                    with eng.Else():
                        arm(3)

        @block.gpsimd
        def _(gpsimd):
            gpsimd.memset(ones.ap(), 1.0).then_inc(gch, 1)

        @block.sync
        def _(sync):
            # inputs: samples 0-2 whole, sample 3 in NCHUNK chunks
            for s in range(SPC - 1):
                sync.dma_start(out=samps[s].ap(), in_=x_aps[s]).then_inc(
                    dmains[s], 16
                )
            for c in range(NCHUNK):
                ch, off = c // 2, (c % 2) * (SP // 2)
                sl = slice(off, off + SP // 2)
                sync.dma_start(
                    out=samps[LAST].ap()[:, ch : ch + 1, sl],
                    in_=x_aps[LAST][:, ch : ch + 1, sl],
                ).then_inc(dmain3s[c], 16)

            # Writes are split across BOTH HWDGE rings and mostly issued
            # EARLY (scalar ring, as each selection lands) so they overlap
            # the read stream instead of bunching into the post-read
            # window: all 8 cores' write phases coincide in wall time and
            # oversubscribe chip HBM, and core 0 systematically loses that
            # arbitration (~176 GB/s write crawl, +10us). Only ~2MB stays
            # queued at the tail of the sync ring (out2 + out3-plane0) —
            # enough to keep the pipe busy while sample 3's argmax decision
            # and copy resolve, small enough to cap the straggler's pain.
            sync.wait_ge(vch, VCH["sel2"])
            sync.dma_start(out=out_aps[2], in_=obufs[2].ap()).then_inc(
                doutA, 16
            )
            sync.wait_ge(vch, VCH["sel3"])
            sync.dma_start(
                out=out_aps[LAST][:, 0], in_=obufs[LAST].ap()[:, 0]
            ).then_inc(doutA, 16)

            sync.wait_ge(doutA, 32)
            sync.wait_ge(doutB, 48)

        @block.tensor
        def _(tensor):
            tensor.wait_ge(gch, 1)
            for s in range(SPC - 1):
                tensor.wait_ge(sch, SCH[f"n{s}"])
                tensor.wait_ge(vch, VCH[f"n{s}"])
                tensor.matmul(
                    psums[s].ap(),
                    ones.ap(),
                    npart.ap()[:, 4 * s : 4 * s + 4],
                    start=True,
                    stop=True,
                ).then_inc(mm, 1)
            mv = npart.ap()[:, 16:32].rearrange("p (k c) -> p c k", c=NCHUNK)
            for c in range(NCHUNK):
                tensor.wait_ge(sch, SCH[f"nc{c}"])
                tensor.wait_ge(vch, VCH[f"nc{c}"])
                tensor.matmul(
                    psums[LAST].ap(),
                    ones.ap(),
                    mv[:, c],
                    start=(c == 0),
                    stop=(c == NCHUNK - 1),
                ).then_inc(mm, 1)

        @block.scalar
        def _(scalar):
            cnt = [0]

            def emit(inst):
                inst.then_inc(sch, 1)
                cnt[0] += 1

            def barrier():
                if cnt[0]:
                    scalar.wait_ge(sch, cnt[0])

            # preload the Square activation table before any data arrives
            emit(
                scalar.activation(
                    sdump.ap()[:, 0:1], sdump.ap()[:, 0:1], ACT.Square, scale=0.0
                )
            )

            # norms k=0,1; each activation lowers to ACT + READ_ACCUM.
            # Between samples, flush the previous sample's output write on
            # the scalar HWDGE ring: the sel wait is long satisfied by the
            # time the next sample's input lands, so this never stalls the
            # norm pipeline, and the write drains concurrently with the
            # read stream.
            for s in range(SPC - 1):
                scalar.wait_ge(dmains[s], 16)
                for k in (0, 1):
                    barrier()
                    emit(
                        scalar.activation(
                            zsink(2 * H2), V(s, k), ACT.Square,
                            accum_out=ncol_s(s, k),
                        )
                    )
                if s <= 1:
                    # flush this sample's output on the scalar ring as soon
                    # as its selection lands (sel_s trails our norms by
                    # ~2us, well before the next sample's input arrives)
                    scalar.wait_ge(vch, VCH[f"sel{s}"])
                    scalar.dma_start(
                        out=out_aps[s], in_=obufs[s].ap()
                    ).then_inc(doutB, 16)
            for c in range(NCHUNK):
                scalar.wait_ge(dmain3s[c], 16)
                for k in (0, 1):
                    barrier()
                    emit(
                        scalar.activation(
                            zsink(16), Vc(LAST, k, c), ACT.Square,
                            accum_out=ncol_c(k, c),
                        )
                    )

            # s3 plane 1 output rides the scalar ring (vector copies the
            # whole sample; an ACT-copy here would trigger a conservative
            # ACT_TABLE_LOAD inside the branch arms, right in the tail)
            scalar.wait_ge(vch, VCH["sel3"])
            scalar.dma_start(
                out=out_aps[LAST][:, 1], in_=obufs[LAST].ap()[:, 1]
            ).then_inc(doutB, 16)

        @block.vector
        def _(vector):
            cnt = [0]

            def emit(inst):
                inst.then_inc(vch, 1)
                cnt[0] += 1

            def barrier():
                if cnt[0]:
                    vector.wait_ge(vch, cnt[0])

            regs = [ctx.enter_context(vector.register(f"ve_r{i}"))
                    for i in range(5)]

            def ttr(out_sink, in_, acc):
                # out = (in*1.0)*in = in^2, accum_out = per-partition sum
                emit(
                    vector.scalar_tensor_tensor(
                        out=out_sink, in0=in_, scalar=1.0, in1=in_,
                        op0=OP.mult, op1=OP.mult, accum_out=acc,
                    )
                )

            def dsink(k, n):
                return bass.AP(dumps[k - 2], 0, [[2 * OSP, 128], [W2, n], [1, W2]])

            def nrm(s):
                vector.wait_ge(dmains[s], 16)
                barrier()  # prior group's writes to the dump sinks
                for k in (2, 3):
                    ttr(dsink(k, 2 * H2), V(s, k), ncol_s(s, k))

            def nrm_c(c):
                vector.wait_ge(dmain3s[c], 16)
                barrier()
                for k in (2, 3):
                    ttr(dsink(k, 16), Vc(LAST, k, c), ncol_c(k, c))

            def cp(s):
                vector.wait_ge(mm, MM[f"s{s}"])
                emit(
                    vector.tensor_copy(
                        nsum.ap()[0:1, 4 * s : 4 * s + 4], psums[s].ap()[0:1]
                    )
                )

            def sel(s, plane=None):
                barrier()  # nsum must be drained before the TENSOR_LOAD
                load_max(vector, s, regs)

                def arm(k):
                    # exactly one arm runs at runtime; +1 on vch either way
                    vector.tensor_copy(
                        OB(s, plane=plane), V(s, k, plane=plane)
                    ).then_inc(vch, 1)

                branch_select(vector, regs, arm)
                cnt[0] += 1
                assert cnt[0] == VCH[f"sel{s}"]

            nrm(0)
            cp(0)
            sel(0)
            nrm(1)
            cp(1)
            sel(1)
            nrm(2)
            nrm_c(0)
            cp(2)
            sel(2)
            for c in range(1, NCHUNK):
                nrm_c(c)
            cp(3)
            sel(3)  # plane 1 is scalar's
            assert cnt[0] == VCH_TOTAL

    return nc


_NC_CACHE = None


def _get_nc():
    global _NC_CACHE
    if _NC_CACHE is None:
        _NC_CACHE = build_nc()
    return _NC_CACHE


def _ensure_devices():
    """Best-effort: make sure the axon NeuronCore backend is selected even if
    the caller initialized jax with a CPU-only platform."""
    import jax

    try:
        if len(jax.devices()) >= NCORES:
            return
    except Exception:
        pass
    try:
        jax.config.update("jax_platforms", "axon")
    except Exception:
        pass


def kernel(x) -> np.ndarray:
    _ensure_devices()
    x = np.asarray(x, dtype=np.float32)
    assert x.shape == (B, C, H, W), x.shape
    shards = np.split(x, NCORES, axis=0)
    in_maps = [{"x": s} for s in shards]
    res = run_bass_kernel_spmd(_get_nc(), in_maps, core_ids=list(range(NCORES)))
    return np.concatenate([r["out"] for r in res.results], axis=0)

